# revision 1
# baseline (speedup 1.0000x reference)
"""COMA loss kernel for Trainium2 (8 NeuronCores, data-parallel over batch).

Reference computation (see problem): given logit/q_value/target_q_value
(T,B,A,N), action (T,B,A), reward (T,B), weight (T,B,A), compute
(policy_loss, q_value_loss, entropy_loss) scalars.

Sharding: B=128 split 8 ways -> B_local=16 per core. Per core the
(b,a) plane is 16*8 = 128 rows = exactly the SBUF partition count, so
all per-(t,b,a) quantities live as [128, T] tiles and the lambda-return
time recurrence runs as a single hardware scan instruction per core.
Each core emits per-partition partial sums [128,3]; the host adds them
up and divides by the global element counts (the "all-reduce of three
scalar means").
"""

import sys

for _p in ("/opt/trn_rl_repo",):
    if _p not in sys.path:
        sys.path.insert(0, _p)

import numpy as np

import concourse.bass as bass
import concourse.bacc as bacc
import concourse.mybir as mybir
from concourse.bass_utils import run_bass_kernel_spmd
from concourse.tile import TileContext

T, B, A, N = 256, 128, 8, 64
M = 8                 # cores
BL = B // M           # local batch
BA = BL * A           # 128 rows -> partition dim
TC = 32               # t-chunk size
GAMMA, LAMBDA = 0.99, 0.95

F32 = mybir.dt.float32


def build_program() -> bass.Bass:
    nc = bacc.Bacc("TRN2", target_bir_lowering=False, debug=False)

    logit = nc.dram_tensor("logit", [T, BA, N], F32, kind="ExternalInput")
    qv = nc.dram_tensor("qv", [T, BA, N], F32, kind="ExternalInput")
    tqv = nc.dram_tensor("tqv", [T, BA, N], F32, kind="ExternalInput")
    act = nc.dram_tensor("act", [BA, T], F32, kind="ExternalInput")
    wgt = nc.dram_tensor("wgt", [BA, T], F32, kind="ExternalInput")
    rwd = nc.dram_tensor("rwd", [BA, T], F32, kind="ExternalInput")
    out = nc.dram_tensor("out", [BA, 3], F32, kind="ExternalOutput")

    AX = mybir.AxisListType.X
    OP = mybir.AluOpType

    with TileContext(nc) as tc:
        with (
            tc.tile_pool(name="inp", bufs=2) as inp,
            tc.tile_pool(name="scr", bufs=2) as scr,
            tc.tile_pool(name="per", bufs=1) as per,
        ):
            # constants / small inputs
            iota_i = per.tile([BA, N], mybir.dt.int32)
            nc.gpsimd.iota(iota_i[:], pattern=[[1, N]], base=0, channel_multiplier=0)
            iota_f = per.tile([BA, N], F32)
            nc.vector.tensor_copy(iota_f[:], iota_i[:])

            act_t = per.tile([BA, T], F32)
            nc.sync.dma_start(out=act_t[:], in_=act[:])
            w_t = per.tile([BA, T], F32)
            nc.sync.dma_start(out=w_t[:], in_=wgt[:])
            r_t = per.tile([BA, T], F32)
            nc.sync.dma_start(out=r_t[:], in_=rwd[:])

            # per-(t,ba) scalar accumulators, [128, T]
            sum_e = per.tile([BA, T], F32)
            dot_eq = per.tile([BA, T], F32)
            dot_el = per.tile([BA, T], F32)
            q_tk = per.tile([BA, T], F32)
            tq_tk = per.tile([BA, T], F32)
            l_tk = per.tile([BA, T], F32)

            # ---- stage 1: streamed over t-chunks -------------------------
            for c in range(T // TC):
                t0 = c * TC
                sl = slice(t0, t0 + TC)

                lg = inp.tile([BA, TC, N], F32, tag="lg")
                qt = inp.tile([BA, TC, N], F32, tag="qt")
                tq = inp.tile([BA, TC, N], F32, tag="tq")
                nc.sync.dma_start(out=lg[:], in_=logit[sl].transpose([1, 0, 2]))
                nc.sync.dma_start(out=qt[:], in_=qv[sl].transpose([1, 0, 2]))
                nc.sync.dma_start(out=tq[:], in_=tqv[sl].transpose([1, 0, 2]))

                # e = exp(logit): |logit| <= ~6 so no max-subtraction needed
                e = scr.tile([BA, TC, N], F32, tag="e")
                nc.scalar.activation(
                    out=e[:], in_=lg[:], func=mybir.ActivationFunctionType.Exp
                )
                nc.vector.reduce_sum(out=sum_e[:, sl], in_=e[:], axis=AX)

                # onehot over the action index
                oh = scr.tile([BA, TC, N], F32, tag="oh")
                nc.vector.tensor_tensor(
                    out=oh[:],
                    in0=iota_f[:].unsqueeze(1).to_broadcast([BA, TC, N]),
                    in1=act_t[:, sl].unsqueeze(2).to_broadcast([BA, TC, N]),
                    op=OP.is_equal,
                )

                pr = scr.tile([BA, TC, N], F32, tag="pr")
                nc.vector.tensor_mul(pr[:], e[:], qt[:])
                nc.vector.reduce_sum(out=dot_eq[:, sl], in_=pr[:], axis=AX)
                nc.vector.tensor_mul(pr[:], e[:], lg[:])
                nc.vector.reduce_sum(out=dot_el[:, sl], in_=pr[:], axis=AX)
                nc.vector.tensor_mul(pr[:], oh[:], qt[:])
                nc.vector.reduce_sum(out=q_tk[:, sl], in_=pr[:], axis=AX)
                nc.vector.tensor_mul(pr[:], oh[:], tq[:])
                nc.vector.reduce_sum(out=tq_tk[:, sl], in_=pr[:], axis=AX)
                nc.vector.tensor_mul(pr[:], oh[:], lg[:])
                nc.vector.reduce_sum(out=l_tk[:, sl], in_=pr[:], axis=AX)

            # ---- stage 2: per-(t,ba) scalar math on [128, T] -------------
            z = per.tile([BA, T], F32)  # logsumexp
            nc.scalar.activation(
                out=z[:], in_=sum_e[:], func=mybir.ActivationFunctionType.Ln
            )
            rs = per.tile([BA, T], F32)  # 1/sum_e
            nc.vector.reciprocal(rs[:], sum_e[:])

            logp = per.tile([BA, T], F32)
            nc.vector.tensor_tensor(out=logp[:], in0=l_tk[:], in1=z[:], op=OP.subtract)
            bl = per.tile([BA, T], F32)  # baseline = dot_eq / sum_e
            nc.vector.tensor_mul(bl[:], dot_eq[:], rs[:])
            adv = per.tile([BA, T], F32)
            nc.vector.tensor_tensor(out=adv[:], in0=q_tk[:], in1=bl[:], op=OP.subtract)
            ent = per.tile([BA, T], F32)  # entropy = z - dot_el / sum_e
            nc.vector.tensor_mul(ent[:], dot_el[:], rs[:])
            nc.vector.tensor_tensor(out=ent[:], in0=z[:], in1=ent[:], op=OP.subtract)

            pol = per.tile([BA, T], F32)  # logp * adv * w
            nc.vector.tensor_mul(pol[:], logp[:], adv[:])
            nc.vector.tensor_mul(pol[:], pol[:], w_t[:])
            entw = per.tile([BA, T], F32)
            nc.vector.tensor_mul(entw[:], ent[:], w_t[:])

            # lambda returns: ret[t] = d[t] + g*l*ret[t+1], scanned in
            # reverse time via negative-step views.
            # d[t] = reward[t] + g*(1-l)*tq_taken[t+1], t in [0, T-2];
            # initial state tq_taken[T-1] makes ret[T-2] = reward[T-2] +
            # g*tq_taken[T-1] as required.
            d = per.tile([BA, T - 1], F32)
            nc.vector.tensor_scalar_mul(d[:], tq_tk[:, 1:T], GAMMA * (1.0 - LAMBDA))
            nc.vector.tensor_add(d[:], d[:], r_t[:, 0 : T - 1])
            gl = per.tile([BA, 1], F32)
            nc.vector.memset(gl[:], GAMMA * LAMBDA)
            ret = per.tile([BA, T - 1], F32)
            nc.vector.tensor_tensor_scan(
                out=ret[:, ::-1],
                data0=gl[:].to_broadcast([BA, T - 1]),
                data1=d[:, ::-1],
                initial=tq_tk[:, T - 1 : T],
                op0=OP.mult,
                op1=OP.add,
            )

            qd = per.tile([BA, T - 1], F32)
            nc.vector.tensor_tensor(
                out=qd[:], in0=ret[:], in1=q_tk[:, 0 : T - 1], op=OP.subtract
            )
            nc.vector.tensor_mul(qd[:], qd[:], qd[:])
            nc.vector.tensor_mul(qd[:], qd[:], w_t[:, 0 : T - 1])

            partials = per.tile([BA, 3], F32)
            nc.vector.reduce_sum(out=partials[:, 0:1], in_=pol[:], axis=AX)
            nc.vector.reduce_sum(out=partials[:, 1:2], in_=qd[:], axis=AX)
            nc.vector.reduce_sum(out=partials[:, 2:3], in_=entw[:], axis=AX)
            nc.sync.dma_start(out=out[:], in_=partials[:])

    return nc


def make_in_maps(logit, action, q_value, target_q_value, reward, weight):
    """Shard + marshal full inputs into per-core input dicts."""
    logit = np.asarray(logit, np.float32)
    q_value = np.asarray(q_value, np.float32)
    target_q_value = np.asarray(target_q_value, np.float32)
    action = np.asarray(action)
    reward = np.asarray(reward, np.float32)
    weight = np.asarray(weight, np.float32)

    in_maps = []
    for r in range(M):
        bs, be = r * BL, (r + 1) * BL
        in_maps.append(
            {
                "logit": np.ascontiguousarray(logit[:, bs:be].reshape(T, BA, N)),
                "qv": np.ascontiguousarray(q_value[:, bs:be].reshape(T, BA, N)),
                "tqv": np.ascontiguousarray(
                    target_q_value[:, bs:be].reshape(T, BA, N)
                ),
                "act": np.ascontiguousarray(
                    action[:, bs:be].reshape(T, BA).T.astype(np.float32)
                ),
                "wgt": np.ascontiguousarray(weight[:, bs:be].reshape(T, BA).T),
                "rwd": np.ascontiguousarray(
                    np.repeat(reward[:, bs:be], A, axis=1).T
                ),
            }
        )
    return in_maps


def combine_partials(partials_per_core):
    """[M][128,3] partial sums -> the three scalar losses."""
    s = np.stack(partials_per_core).astype(np.float64).sum(axis=(0, 1))
    policy_loss = np.float32(-s[0] / (T * B * A))
    q_value_loss = np.float32(s[1] / ((T - 1) * B * A))
    entropy_loss = np.float32(s[2] / (T * B * A))
    return policy_loss, q_value_loss, entropy_loss


_program_cache = {}


def _get_program() -> bass.Bass:
    if "nc" not in _program_cache:
        nc = build_program()
        nc.finalize()
        _program_cache["nc"] = nc
    return _program_cache["nc"]


def kernel(logit, action, q_value, target_q_value, reward, weight):
    nc = _get_program()
    in_maps = make_in_maps(logit, action, q_value, target_q_value, reward, weight)
    res = run_bass_kernel_spmd(nc, in_maps, list(range(M))).results
    return combine_partials([np.asarray(res[i]["out"]) for i in range(M)])

